# revision 26
# baseline (speedup 1.0000x reference)
"""LoRA attention processor on 8 NeuronCores (Trainium2, Bass/Tile), bf16.

Reference computation (B=2, S=4096, D=1280, H=8 heads, dh=160, rank-4 LoRA
on K/V):
    q = x @ Wq; k = x @ Wk; v = x @ Wv
    k += (k @ Ak) @ Bk; v += (v @ Av) @ Bv        (LoRA, rank 4)
    attn = softmax(q k^T / sqrt(dh)) v   per head
    out = attn @ Wout + b_out

Sharding: core c handles batch b = c//4 and head pair p = c%4 (columns
320p:320p+320 of the QKV projections, rows of Wout). LoRA is folded into
the weights on the host. Each core returns a partial output (its heads'
contribution to attn@Wout); the host sums the 4 partials per batch and
adds the bias.

Design notes (all empirically driven; the PE gets power-clamped to
1.2 GHz after ~220us of sustained 8-core matmul activity, so the kernel
is column-count-bound):
- All matmuls in bf16 (fp32r keeps the clamp at ~50% util for the whole
  run); everything SBUF-resident.
- Projections: per-head d0:128 chunks of Q and K are produced as [128,S]
  tiles; the two heads' leftover d128:160 dims of BOTH q and k are fused
  into one 128-wide M-chunk (host column permutation), then a SBUF->SBUF
  DMA builds a half-swapped duplicate so that k3/q3 coexist in strips
  {32h} and {64+32h} - the two K=32 leftover score matmuls of a q-chunk
  then run CONCURRENTLY on disjoint 32-row PE tiles (different PSUM
  banks; same-bank concurrency is a hardware fault).
- Scores are computed transposed ([k,q]) in 1024-wide q chunks, j-pairs
  batched so 128-row-mode and 32-row-mode matmuls alternate once per j
  instead of twice (mode switches drain the PE). exp runs on ACT over
  [128,1024] (2 PSUM banks per instruction, halving ACT overhead).
- PV runs in natural layout out[q,d]: lhsT = exp-tile slices, rhs =
  V[k-block] with a ones-column appended (denominator rides along as
  column 160). PSUM start=True clears has_written bits BANK-wide, so only
  the first matmul of each bank uses start=True; the other interleaved
  accumulation groups in that bank begin with start=False (cleared bits
  -> overwrite) - verified on hardware.
- Normalization is folded into PSUM evacuation (ACT copy with
  per-partition 1/denom scale), the normalized tiles are transposed on
  the PE, and the output projection contracts d in (128,128,64) chunks.
"""

import numpy as np
import ml_dtypes
from contextlib import ExitStack

import concourse.bass as bass
import concourse.tile as tile
from concourse import bacc, mybir
from concourse.bass_utils import run_bass_kernel_spmd

B, S, D = 2, 4096, 1280
H, DH = 8, 160
HP = 320           # head-pair columns per core (2 heads)
N_CORES = 8
SC = 512           # phase-1 free-dim chunk
NSC = S // SC      # 8
QC = 1024          # phase-2 q chunk (2 PSUM banks wide)
NQC = S // QC      # 4
CK = 128           # contraction chunk
NCK = D // CK      # 10
NJ = S // 128      # 32 k-blocks
F32 = mybir.dt.float32
BF16 = mybir.dt.bfloat16
EXP = mybir.ActivationFunctionType.Exp

CHUNKS = [(0, 128), (128, 128), (256, 64)]   # oT / wo row chunks

_CACHE = {}


def build():
    nc = bacc.Bacc("TRN2", target_bir_lowering=False, debug=False,
                   num_devices=N_CORES)
    xT = nc.dram_tensor("xT", [D, S], BF16, kind="ExternalInput").ap()
    wq = nc.dram_tensor("wq", [D, 256], BF16, kind="ExternalInput").ap()
    wk = nc.dram_tensor("wk", [D, 256], BF16, kind="ExternalInput").ap()
    wqk = nc.dram_tensor("wqk", [D, 128], BF16, kind="ExternalInput").ap()
    wv = nc.dram_tensor("wv", [D, HP], BF16, kind="ExternalInput").ap()
    wo = nc.dram_tensor("wo", [HP, D], BF16, kind="ExternalInput").ap()
    ident = nc.dram_tensor("ident", [128, 128], BF16, kind="ExternalInput").ap()
    out = nc.dram_tensor("out", [S, D], F32, kind="ExternalOutput").ap()

    with tile.TileContext(nc) as tc, ExitStack() as top:
        # persistent SBUF tensors
        per = top.enter_context(tc.tile_pool(name="per", bufs=1))
        qt = [per.tile([128, S], BF16, name=f"qt{i}", tag=f"qt{i}")
              for i in range(2)]
        kt = [per.tile([128, S], BF16, name=f"kt{i}", tag=f"kt{i}")
              for i in range(2)]
        # leftover-dim tiles; rows hold [q3 h0|q3 h1|k3 h0|k3 h1] and the
        # swapped/head-swapped variants so each head's (k3, q3) pair exists
        # in all four 32-partition strips (4-way concurrent K=32 matmuls)
        qkb = per.tile([128, S], BF16, name="qkb", tag="qkb")
        qkd = per.tile([128, S], BF16, name="qkd", tag="qkd")
        qkb2 = per.tile([128, S], BF16, name="qkb2", tag="qkb2")
        qkd2 = per.tile([128, S], BF16, name="qkd2", tag="qkd2")
        V = [per.tile([128, NJ, 162], BF16, name=f"V{h}", tag=f"V{h}")
             for h in range(2)]
        oT = [per.tile([sz, S], BF16, name=f"oT{i}", tag=f"oT{i}")
              for i, (_, sz) in enumerate(CHUNKS)]
        id_t = per.tile([128, 128], BF16, name="id_t", tag="id_t")

        # ---- phase 1: projections Q/K (transposed chunks) + V (natural) ----
        with ExitStack() as ph1:
            xp = ph1.enter_context(tc.tile_pool(name="xp", bufs=2))
            wp = ph1.enter_context(tc.tile_pool(name="wp", bufs=1))
            pp = ph1.enter_context(tc.tile_pool(name="pp", bufs=4, space="PSUM"))
            sp = ph1.enter_context(tc.tile_pool(name="sp", bufs=2))

            warm = sp.tile([1, 2], F32, tag="warm")
            nc.vector.memset(warm[:], 0.0)
            warm2 = sp.tile([1, 2], F32, tag="warm2")
            nc.scalar.activation(warm2[:], warm[:], EXP)

            # one strided DMA per weight tensor: dram rows c*128+p ->
            # partition p, chunk c of a [128, NCK, w] tile. wq and the first
            # x chunk are issued first so the very first matmul can start.
            wts = {}
            xt0 = None
            for nm, srcw, w_ in (("wq", wq, 256), ("wk", wk, 256),
                                 ("wqk", wqk, 128), ("wv", wv, HP)):
                t = wp.tile([128, NCK, w_], BF16, name=f"w_{nm}",
                            tag=f"w_{nm}")
                nc.sync.dma_start(
                    t[:], srcw.rearrange("(c p) w -> p c w", p=CK))
                for c in range(NCK):
                    wts[(nm, c)] = t[:, c]
                if nm == "wq":
                    xt0 = xp.tile([CK, NCK, SC], BF16, tag="xt", name="xt")
                    nc.sync.dma_start(
                        xt0[:], xT.rearrange("(c p) s -> p c s",
                                             p=CK)[:, :, 0:SC])
            nc.sync.dma_start(id_t[:], ident[:])
            for h in range(2):
                nc.vector.memset(V[h][:, :, 160:162], 0.0)
                nc.vector.memset(V[h][:, :, 160:161], 1.0)

            for sc in range(NSC):
                ss = slice(sc * SC, (sc + 1) * SC)
                if sc == 0:
                    xt = xt0
                else:
                    xt = xp.tile([CK, NCK, SC], BF16, tag="xt", name="xt")
                    nc.sync.dma_start(
                        xt[:], xT.rearrange("(c p) s -> p c s",
                                            p=CK)[:, :, ss])
                xts = [xt[:, c] for c in range(NCK)]
                # transposed projections: psum[m, s] = w[c, m].T @ xT[c, s]
                groups = [("wq", 0, qt[0]), ("wq", 128, qt[1]),
                          ("wk", 0, kt[0]), ("wk", 128, kt[1]),
                          ("wqk", 0, qkb)]
                for nm, off, dst in groups:
                    ps = pp.tile([128, SC], F32, tag="ps")
                    for c in range(NCK):
                        nc.tensor.matmul(
                            ps[:], wts[(nm, c)][:, off:off + 128], xts[c][:],
                            start=(c == 0), stop=(c == NCK - 1))
                    nc.vector.tensor_copy(dst[:, ss], ps[:])
                    if dst is qkb:
                        # duplicates via SBUF->SBUF DMA (shifts partitions),
                        # chunk-wise so they overlap phase-1 compute:
                        # qkb  = [q3h0|q3h1|k3h0|k3h1]   (produced)
                        # qkd  = [k3h0|k3h1|q3h0|q3h1]   (half swap)
                        # qkb2 = [q3h1|q3h0|k3h1|k3h0]   (head swap)
                        # qkd2 = [k3h1|k3h0|q3h1|q3h0]   (both)
                        nc.sync.dma_start(qkd[0:64, ss], qkb[64:128, ss])
                        nc.sync.dma_start(qkd[64:128, ss], qkb[0:64, ss])
                        for dst2, srcs in ((qkb2, (32, 0, 96, 64)),
                                           (qkd2, (96, 64, 32, 0))):
                            for r, sr in enumerate(srcs):
                                nc.sync.dma_start(
                                    dst2[32 * r:32 * r + 32, ss],
                                    qkb[sr:sr + 32, ss])
                # V natural: psum[s, dv] = xT[c, s].T @ wv[c, :]
                for st4 in range(4):
                    s0 = sc * 4 + st4
                    ps = pp.tile([128, HP], F32, tag="psv")
                    for c in range(NCK):
                        nc.tensor.matmul(
                            ps[:], xts[c][:, st4 * 128:(st4 + 1) * 128],
                            wts[("wv", c)][:], start=(c == 0),
                            stop=(c == NCK - 1))
                    for h in range(2):
                        nc.vector.tensor_copy(V[h][:, s0, 0:160],
                                              ps[:, h * 160:(h + 1) * 160])



        # ---- phase 2+3: attention + output projection, per 1024-q chunk ----
        # PSUM budget (8 banks): "sc"-tagged [128,1024]f32 tiles (2 banks x
        # 2 bufs = 4) host the score matmuls AND (via disjoint slices) the
        # transpose outputs and phase-3 accumulators; pv tiles take 3 banks.
        with ExitStack() as ph2:
            big = ph2.enter_context(tc.tile_pool(name="big", bufs=2,
                                                 space="PSUM"))
            pvp = ph2.enter_context(tc.tile_pool(name="pvp", bufs=1,
                                                 space="PSUM"))
            ptp = ph2.enter_context(tc.tile_pool(name="ptp", bufs=1,
                                                 space="PSUM"))
            exp_ = ph2.enter_context(tc.tile_pool(name="exq", bufs=6))
            nap = ph2.enter_context(tc.tile_pool(name="nap", bufs=2))
            nnp = ph2.enter_context(tc.tile_pool(name="nnp", bufs=2))
            rcp = ph2.enter_context(tc.tile_pool(name="rcp", bufs=2))
            wop = ph2.enter_context(tc.tile_pool(name="wop", bufs=1))
            fsp = ph2.enter_context(tc.tile_pool(name="fsp", bufs=3))

            woc = []
            for i, (off, msz) in enumerate(CHUNKS):
                w = wop.tile([msz, D], BF16, name=f"wo{i}", tag=f"wo{i}")
                nc.sync.dma_start(w[:], wo[off:off + msz, :])
                woc.append(w)

            # pv psum slot for a q-subtile (0..7): 3+3+2 per bank
            def pv_slot(pvt, ql):
                if ql < 3:
                    return pvt[0][:, ql]
                if ql < 6:
                    return pvt[1][:, ql - 3]
                return pvt[2][:, ql - 6]

            def emit_tail(natA, natN, q0, qls):
                # per q-subtile: transpose natural [q, d] -> oT chunks [d, q]
                # (f32 PE transpose mode, own 1-bank pool), then immediately
                # the output projection for that 128-row block. Emitted in
                # two halves at the points where the PE would otherwise stall
                # on the ACT evacuation chain (next chunk's start and its
                # h0->h1 transition).
                for ql in qls:
                    qg = slice(q0 + ql * 128, q0 + (ql + 1) * 128)
                    pt = ptp.tile([128, 3, 128], BF16, tag="pt", name="pt")
                    nc.tensor.transpose(pt[:, 0], natA[(0, ql)][:], id_t[:])
                    nc.tensor.transpose(pt[:, 1], natA[(1, ql)][:], id_t[:])
                    nc.tensor.transpose(pt[0:64, 2], natN[ql][:], id_t[:])
                    nc.vector.tensor_copy(oT[0][:, qg], pt[:, 0])
                    nc.vector.tensor_copy(oT[1][:, qg], pt[:, 1])
                    nc.vector.tensor_copy(oT[2][:, qg], pt[0:64, 2])
                    row = q0 + ql * 128
                    sbs = slice(row, row + 128)
                    fs = fsp.tile([128, D], F32, tag="fs", name="fs")
                    for oi, (oc, osz) in enumerate(((0, 512), (512, 512),
                                                    (1024, 256))):
                        ps = big.tile([128, QC], F32, tag="sc", name="fo")
                        for i in range(3):
                            nc.tensor.matmul(ps[:, 0:osz], oT[i][:, sbs],
                                             woc[i][:, oc:oc + osz],
                                             start=(i == 0), stop=(i == 2))
                        if oi % 2 == 0:
                            nc.vector.tensor_copy(fs[:, oc:oc + osz],
                                                  ps[:, 0:osz])
                        else:
                            nc.scalar.copy(fs[:, oc:oc + osz], ps[:, 0:osz])
                        nc.sync.dma_start(out[sbs, oc:oc + osz],
                                          fs[:, oc:oc + osz])

            pending = None

            for qc in range(NQC):
                q0 = qc * QC
                pairs = [(h, jp) for h in range(2) for jp in range(0, NJ, 2)]
                pvt = [pvp.tile([128, 3, 162], F32, tag="pv0", name="pv0"),
                       pvp.tile([128, 3, 162], F32, tag="pv1", name="pv1"),
                       pvp.tile([128, 2, 162], F32, tag="pv2", name="pv2")]
                exs = {}

                def emit_scores(h, jp):
                    """Scores for j-pair (jp, jp+1): 128-mode matmuls batched
                    before 32-mode ones (1 mode switch per j instead of 2);
                    the two K=32 leftovers of each j run concurrently on
                    disjoint 32-row tiles / different PSUM banks."""
                    scp = [big.tile([128, QC], F32, tag="sc", name="sca"),
                           big.tile([128, QC], F32, tag="sc", name="scb")]
                    for jj in range(2):
                        js = slice((jp + jj) * 128, (jp + jj + 1) * 128)
                        for half in range(2):
                            qs = slice(q0 + half * 512, q0 + (half + 1) * 512)
                            nc.tensor.matmul(
                                scp[jj][:, half * 512:(half + 1) * 512],
                                kt[h][:, js], qt[h][:, qs],
                                start=True, stop=False)
                    # the 4 leftover K=32 matmuls of the pair run on the 4
                    # distinct 32-row strips (4 distinct PSUM banks) -> all
                    # concurrent on the PE
                    for bi in range(4):
                        jj, half = bi // 2, bi % 2
                        s = 32 * bi
                        use2 = (bi + h) % 2 == 1
                        ktile = ((qkd2 if use2 else qkd) if bi < 2
                                 else (qkb2 if use2 else qkb))
                        qtile = ((qkb2 if use2 else qkb) if bi < 2
                                 else (qkd2 if use2 else qkd))
                        js = slice((jp + jj) * 128, (jp + jj + 1) * 128)
                        qs = slice(q0 + half * 512, q0 + (half + 1) * 512)
                        nc.tensor.matmul(
                            scp[jj][:, half * 512:(half + 1) * 512],
                            ktile[s:s + 32, js], qtile[s:s + 32, qs],
                            start=False, stop=True, tile_position=(s, 0))
                    for jj in range(2):
                        ex = exp_.tile([128, QC], BF16, tag="ex")
                        nc.scalar.activation(ex[:], scp[jj][:], EXP)
                        exs[(h, jp + jj)] = ex

                def emit_pv(h, jp):
                    for jj in range(2):
                        j = jp + jj
                        ex = exs.pop((h, j))
                        for ql in range(8):
                            # start=True clears has_written BANK-wide: only
                            # the first matmul per bank may use it.
                            st = (j == 0) and ql in (0, 3, 6)
                            nc.tensor.matmul(
                                pv_slot(pvt, ql),
                                ex[:, ql * 128:(ql + 1) * 128],
                                V[h][:, j, :], start=st, stop=(j == NJ - 1),
                                skip_group_check=True)

                natA = {}
                natN = [nnp.tile([128, 64], BF16, tag=f"nn{ql}",
                                 name=f"nn{ql}")
                        for ql in range(8)]

                def emit_evac(h):
                    for ql in range(8):
                        pv = pv_slot(pvt, ql)
                        rec = rcp.tile([128, 1], F32, tag=f"rc{h}_{ql}",
                                       name="rec")
                        nc.vector.reciprocal(rec[:], pv[:, 160:161])
                        na = nap.tile([128, 128], BF16, tag=f"na{h}_{ql}",
                                      name="na")
                        nc.vector.tensor_scalar_mul(na[:], pv[:, 0:128],
                                                    rec[:])
                        nc.vector.tensor_scalar_mul(
                            natN[ql][:, 32 * h:32 * h + 32],
                            pv[:, 128:160], rec[:])
                        natA[(h, ql)] = na

                emit_scores(*pairs[0])
                if pending is not None:
                    emit_tail(*pending, range(0, 4))
                for i, (h, jp) in enumerate(pairs):
                    if i + 1 < len(pairs):
                        emit_scores(*pairs[i + 1])
                    emit_pv(h, jp)
                    if jp == NJ - 2:
                        emit_evac(h)
                        if h == 0 and pending is not None:
                            emit_tail(*pending, range(4, 8))
                pending = (natA, natN, q0)
            emit_tail(*pending, range(0, 8))

    nc.compile()
    return nc


def kernel(hidden_states, w_q, w_k, w_v, lora_k_a, lora_k_b,
           lora_v_a, lora_v_b, w_out, b_out):
    f64 = np.float64
    bf16 = ml_dtypes.bfloat16
    wk_eff = (w_k.astype(f64)
              + w_k.astype(f64) @ lora_k_a.astype(f64) @ lora_k_b.astype(f64)
              ).astype(np.float32)
    wv_eff = (w_v.astype(f64)
              + w_v.astype(f64) @ lora_v_a.astype(f64) @ lora_v_b.astype(f64)
              ).astype(np.float32)
    wq_s = (w_q.astype(f64) / np.sqrt(DH)).astype(np.float32)

    ident = np.eye(128, dtype=bf16)
    xT = [np.ascontiguousarray(np.asarray(hidden_states)[b].T).astype(bf16)
          for b in range(B)]

    in_maps = []
    for c in range(N_CORES):
        b, p = c // 4, c % 4
        ha, hb = 2 * p, 2 * p + 1
        mainq = np.concatenate([np.arange(ha * DH, ha * DH + 128),
                                np.arange(hb * DH, hb * DH + 128)])
        left = np.concatenate([np.arange(ha * DH + 128, (ha + 1) * DH),
                               np.arange(hb * DH + 128, (hb + 1) * DH)])
        # wo rows follow the oT layout: [h0 d0:128 | h1 d0:128 | leftovers]
        perm = np.concatenate([mainq, left])
        cols = slice(p * HP, (p + 1) * HP)
        in_maps.append({
            "xT": xT[b],
            "wq": np.ascontiguousarray(wq_s[:, mainq]).astype(bf16),
            "wk": np.ascontiguousarray(wk_eff[:, mainq]).astype(bf16),
            "wqk": np.ascontiguousarray(
                np.concatenate([wq_s[:, left], wk_eff[:, left]],
                               axis=1)).astype(bf16),
            "wv": np.ascontiguousarray(wv_eff[:, cols]).astype(bf16),
            "wo": np.ascontiguousarray(w_out[perm, :]).astype(bf16),
            "ident": ident,
        })

    global _last_in_maps
    _last_in_maps = in_maps
    if "nc" not in _CACHE:
        _CACHE["nc"] = build()
    res = run_bass_kernel_spmd(_CACHE["nc"], in_maps, list(range(N_CORES)))

    out = np.zeros((B, S, D), np.float32)
    for c in range(N_CORES):
        out[c // 4] += res.results[c]["out"]
    out += np.asarray(b_out, np.float32)
    return out


# revision 27
# speedup vs baseline: 1.0096x; 1.0096x over previous
"""LoRA attention processor on 8 NeuronCores (Trainium2, Bass/Tile), bf16.

Reference computation (B=2, S=4096, D=1280, H=8 heads, dh=160, rank-4 LoRA
on K/V):
    q = x @ Wq; k = x @ Wk; v = x @ Wv
    k += (k @ Ak) @ Bk; v += (v @ Av) @ Bv        (LoRA, rank 4)
    attn = softmax(q k^T / sqrt(dh)) v   per head
    out = attn @ Wout + b_out

Sharding: core c handles batch b = c//4 and head pair p = c%4 (columns
320p:320p+320 of the QKV projections, rows of Wout). LoRA is folded into
the weights on the host. Each core returns a partial output (its heads'
contribution to attn@Wout); the host sums the 4 partials per batch and
adds the bias.

Design notes (all empirically driven; the PE gets power-clamped to
1.2 GHz after ~220us of sustained 8-core matmul activity, so the kernel
is column-count-bound):
- All matmuls in bf16 (fp32r keeps the clamp at ~50% util for the whole
  run); everything SBUF-resident.
- Projections: per-head d0:128 chunks of Q and K are produced as [128,S]
  tiles; the two heads' leftover d128:160 dims of BOTH q and k are fused
  into one 128-wide M-chunk (host column permutation), then a SBUF->SBUF
  DMA builds a half-swapped duplicate so that k3/q3 coexist in strips
  {32h} and {64+32h} - the two K=32 leftover score matmuls of a q-chunk
  then run CONCURRENTLY on disjoint 32-row PE tiles (different PSUM
  banks; same-bank concurrency is a hardware fault).
- Scores are computed transposed ([k,q]) in 1024-wide q chunks, j-pairs
  batched so 128-row-mode and 32-row-mode matmuls alternate once per j
  instead of twice (mode switches drain the PE). exp runs on ACT over
  [128,1024] (2 PSUM banks per instruction, halving ACT overhead).
- PV runs in natural layout out[q,d]: lhsT = exp-tile slices, rhs =
  V[k-block] with a ones-column appended (denominator rides along as
  column 160). PSUM start=True clears has_written bits BANK-wide, so only
  the first matmul of each bank uses start=True; the other interleaved
  accumulation groups in that bank begin with start=False (cleared bits
  -> overwrite) - verified on hardware.
- Normalization is folded into PSUM evacuation (ACT copy with
  per-partition 1/denom scale), the normalized tiles are transposed on
  the PE, and the output projection contracts d in (128,128,64) chunks.
"""

import numpy as np
import ml_dtypes
from contextlib import ExitStack

import concourse.bass as bass
import concourse.tile as tile
from concourse import bacc, mybir
from concourse.bass_utils import run_bass_kernel_spmd

B, S, D = 2, 4096, 1280
H, DH = 8, 160
HP = 320           # head-pair columns per core (2 heads)
N_CORES = 8
SC = 512           # phase-1 free-dim chunk
NSC = S // SC      # 8
QC = 1024          # phase-2 q chunk (2 PSUM banks wide)
NQC = S // QC      # 4
CK = 128           # contraction chunk
NCK = D // CK      # 10
NJ = S // 128      # 32 k-blocks
F32 = mybir.dt.float32
BF16 = mybir.dt.bfloat16
EXP = mybir.ActivationFunctionType.Exp

CHUNKS = [(0, 128), (128, 128), (256, 64)]   # oT / wo row chunks

_CACHE = {}


def build():
    nc = bacc.Bacc("TRN2", target_bir_lowering=False, debug=False,
                   num_devices=N_CORES)
    xT = nc.dram_tensor("xT", [D, S], BF16, kind="ExternalInput").ap()
    wq = nc.dram_tensor("wq", [D, 256], BF16, kind="ExternalInput").ap()
    wk = nc.dram_tensor("wk", [D, 256], BF16, kind="ExternalInput").ap()
    wqk = nc.dram_tensor("wqk", [D, 128], BF16, kind="ExternalInput").ap()
    wv = nc.dram_tensor("wv", [D, HP], BF16, kind="ExternalInput").ap()
    wo = nc.dram_tensor("wo", [HP, D], BF16, kind="ExternalInput").ap()
    ident = nc.dram_tensor("ident", [128, 128], F32, kind="ExternalInput").ap()
    out = nc.dram_tensor("out", [S, D], F32, kind="ExternalOutput").ap()

    with tile.TileContext(nc) as tc, ExitStack() as top:
        # persistent SBUF tensors
        per = top.enter_context(tc.tile_pool(name="per", bufs=1))
        qt = [per.tile([128, S], BF16, name=f"qt{i}", tag=f"qt{i}")
              for i in range(2)]
        kt = [per.tile([128, S], BF16, name=f"kt{i}", tag=f"kt{i}")
              for i in range(2)]
        # leftover-dim tiles; rows hold [q3 h0|q3 h1|k3 h0|k3 h1] and the
        # swapped/head-swapped variants so each head's (k3, q3) pair exists
        # in all four 32-partition strips (4-way concurrent K=32 matmuls)
        qkb = per.tile([128, S], BF16, name="qkb", tag="qkb")
        qkd = per.tile([128, S], BF16, name="qkd", tag="qkd")
        qkb2 = per.tile([128, S], BF16, name="qkb2", tag="qkb2")
        qkd2 = per.tile([128, S], BF16, name="qkd2", tag="qkd2")
        V = [per.tile([128, NJ, 162], BF16, name=f"V{h}", tag=f"V{h}")
             for h in range(2)]
        oT = [per.tile([sz, S], BF16, name=f"oT{i}", tag=f"oT{i}")
              for i, (_, sz) in enumerate(CHUNKS)]
        id_t = per.tile([128, 128], F32, name="id_t", tag="id_t")

        # ---- phase 1: projections Q/K (transposed chunks) + V (natural) ----
        with ExitStack() as ph1:
            xp = ph1.enter_context(tc.tile_pool(name="xp", bufs=2))
            wp = ph1.enter_context(tc.tile_pool(name="wp", bufs=1))
            pp = ph1.enter_context(tc.tile_pool(name="pp", bufs=4, space="PSUM"))
            sp = ph1.enter_context(tc.tile_pool(name="sp", bufs=2))

            warm = sp.tile([1, 2], F32, tag="warm")
            nc.vector.memset(warm[:], 0.0)
            warm2 = sp.tile([1, 2], F32, tag="warm2")
            nc.scalar.activation(warm2[:], warm[:], EXP)

            # one strided DMA per weight tensor: dram rows c*128+p ->
            # partition p, chunk c of a [128, NCK, w] tile. wq and the first
            # x chunk are issued first so the very first matmul can start.
            wts = {}
            xt0 = None
            for nm, srcw, w_ in (("wq", wq, 256), ("wk", wk, 256),
                                 ("wqk", wqk, 128), ("wv", wv, HP)):
                t = wp.tile([128, NCK, w_], BF16, name=f"w_{nm}",
                            tag=f"w_{nm}")
                nc.sync.dma_start(
                    t[:], srcw.rearrange("(c p) w -> p c w", p=CK))
                for c in range(NCK):
                    wts[(nm, c)] = t[:, c]
                if nm == "wq":
                    xt0 = xp.tile([CK, NCK, SC], BF16, tag="xt", name="xt")
                    nc.sync.dma_start(
                        xt0[:], xT.rearrange("(c p) s -> p c s",
                                             p=CK)[:, :, 0:SC])
            nc.sync.dma_start(id_t[:], ident[:])
            for h in range(2):
                nc.vector.memset(V[h][:, :, 160:162], 0.0)
                nc.vector.memset(V[h][:, :, 160:161], 1.0)

            for sc in range(NSC):
                ss = slice(sc * SC, (sc + 1) * SC)
                if sc == 0:
                    xt = xt0
                else:
                    xt = xp.tile([CK, NCK, SC], BF16, tag="xt", name="xt")
                    nc.sync.dma_start(
                        xt[:], xT.rearrange("(c p) s -> p c s",
                                            p=CK)[:, :, ss])
                xts = [xt[:, c] for c in range(NCK)]
                # transposed projections: psum[m, s] = w[c, m].T @ xT[c, s]
                groups = [("wq", 0, qt[0]), ("wq", 128, qt[1]),
                          ("wk", 0, kt[0]), ("wk", 128, kt[1]),
                          ("wqk", 0, qkb)]
                for nm, off, dst in groups:
                    ps = pp.tile([128, SC], F32, tag="ps")
                    for c in range(NCK):
                        nc.tensor.matmul(
                            ps[:], wts[(nm, c)][:, off:off + 128], xts[c][:],
                            start=(c == 0), stop=(c == NCK - 1))
                    nc.vector.tensor_copy(dst[:, ss], ps[:])
                    if dst is qkb:
                        # duplicates via SBUF->SBUF DMA (shifts partitions),
                        # chunk-wise so they overlap phase-1 compute:
                        # qkb  = [q3h0|q3h1|k3h0|k3h1]   (produced)
                        # qkd  = [k3h0|k3h1|q3h0|q3h1]   (half swap)
                        # qkb2 = [q3h1|q3h0|k3h1|k3h0]   (head swap)
                        # qkd2 = [k3h1|k3h0|q3h1|q3h0]   (both)
                        nc.sync.dma_start(qkd[0:64, ss], qkb[64:128, ss])
                        nc.sync.dma_start(qkd[64:128, ss], qkb[0:64, ss])
                        for dst2, srcs in ((qkb2, (32, 0, 96, 64)),
                                           (qkd2, (96, 64, 32, 0))):
                            for r, sr in enumerate(srcs):
                                nc.sync.dma_start(
                                    dst2[32 * r:32 * r + 32, ss],
                                    qkb[sr:sr + 32, ss])
                # V natural: psum[s, dv] = xT[c, s].T @ wv[c, :]
                for st4 in range(4):
                    s0 = sc * 4 + st4
                    ps = pp.tile([128, HP], F32, tag="psv")
                    for c in range(NCK):
                        nc.tensor.matmul(
                            ps[:], xts[c][:, st4 * 128:(st4 + 1) * 128],
                            wts[("wv", c)][:], start=(c == 0),
                            stop=(c == NCK - 1))
                    for h in range(2):
                        nc.vector.tensor_copy(V[h][:, s0, 0:160],
                                              ps[:, h * 160:(h + 1) * 160])



        # ---- phase 2+3: attention + output projection, per 1024-q chunk ----
        # PSUM budget (8 banks): "sc"-tagged [128,1024]f32 tiles (2 banks x
        # 2 bufs = 4) host the score matmuls AND (via disjoint slices) the
        # transpose outputs and phase-3 accumulators; pv tiles take 3 banks.
        with ExitStack() as ph2:
            big = ph2.enter_context(tc.tile_pool(name="big", bufs=2,
                                                 space="PSUM"))
            pvp = ph2.enter_context(tc.tile_pool(name="pvp", bufs=1,
                                                 space="PSUM"))
            ptp = ph2.enter_context(tc.tile_pool(name="ptp", bufs=1,
                                                 space="PSUM"))
            exp_ = ph2.enter_context(tc.tile_pool(name="exq", bufs=6))
            nap = ph2.enter_context(tc.tile_pool(name="nap", bufs=2))
            nnp = ph2.enter_context(tc.tile_pool(name="nnp", bufs=2))
            rcp = ph2.enter_context(tc.tile_pool(name="rcp", bufs=2))
            wop = ph2.enter_context(tc.tile_pool(name="wop", bufs=1))
            fsp = ph2.enter_context(tc.tile_pool(name="fsp", bufs=2))

            woc = []
            for i, (off, msz) in enumerate(CHUNKS):
                w = wop.tile([msz, D], BF16, name=f"wo{i}", tag=f"wo{i}")
                nc.sync.dma_start(w[:], wo[off:off + msz, :])
                woc.append(w)

            # pv psum slot for a q-subtile (0..7): 3+3+2 per bank
            def pv_slot(pvt, ql):
                if ql < 3:
                    return pvt[0][:, ql]
                if ql < 6:
                    return pvt[1][:, ql - 3]
                return pvt[2][:, ql - 6]

            def emit_tail(natA, natN, q0, qls):
                # per q-subtile: transpose natural [q, d] -> oT chunks [d, q]
                # (f32 PE transpose mode, own 1-bank pool), then immediately
                # the output projection for that 128-row block. Emitted in
                # two halves at the points where the PE would otherwise stall
                # on the ACT evacuation chain (next chunk's start and its
                # h0->h1 transition).
                for ql in qls:
                    qg = slice(q0 + ql * 128, q0 + (ql + 1) * 128)
                    pt = ptp.tile([128, 3, 128], F32, tag="pt", name="pt")
                    nc.tensor.transpose(pt[:, 0], natA[(0, ql)][:], id_t[:])
                    nc.tensor.transpose(pt[:, 1], natA[(1, ql)][:], id_t[:])
                    nc.tensor.transpose(pt[0:64, 2], natN[ql][:], id_t[:])
                    nc.vector.tensor_copy(oT[0][:, qg], pt[:, 0])
                    nc.vector.tensor_copy(oT[1][:, qg], pt[:, 1])
                    nc.vector.tensor_copy(oT[2][:, qg], pt[0:64, 2])
                    row = q0 + ql * 128
                    sbs = slice(row, row + 128)
                    fs = fsp.tile([128, D], F32, tag="fs", name="fs")
                    for oi, (oc, osz) in enumerate(((0, 512), (512, 512),
                                                    (1024, 256))):
                        ps = big.tile([128, QC], F32, tag="sc", name="fo")
                        for i in range(3):
                            nc.tensor.matmul(ps[:, 0:osz], oT[i][:, sbs],
                                             woc[i][:, oc:oc + osz],
                                             start=(i == 0), stop=(i == 2))
                        if oi % 2 == 0:
                            nc.vector.tensor_copy(fs[:, oc:oc + osz],
                                                  ps[:, 0:osz])
                        else:
                            nc.scalar.copy(fs[:, oc:oc + osz], ps[:, 0:osz])
                        nc.sync.dma_start(out[sbs, oc:oc + osz],
                                          fs[:, oc:oc + osz])

            pending = None

            for qc in range(NQC):
                q0 = qc * QC
                pairs = [(h, jp) for h in range(2) for jp in range(0, NJ, 2)]
                pvt = [pvp.tile([128, 3, 162], F32, tag="pv0", name="pv0"),
                       pvp.tile([128, 3, 162], F32, tag="pv1", name="pv1"),
                       pvp.tile([128, 2, 162], F32, tag="pv2", name="pv2")]
                exs = {}

                def emit_scores(h, jp):
                    """Scores for j-pair (jp, jp+1): 128-mode matmuls batched
                    before 32-mode ones (1 mode switch per j instead of 2);
                    the two K=32 leftovers of each j run concurrently on
                    disjoint 32-row tiles / different PSUM banks."""
                    scp = [big.tile([128, QC], F32, tag="sc", name="sca"),
                           big.tile([128, QC], F32, tag="sc", name="scb")]
                    for jj in range(2):
                        js = slice((jp + jj) * 128, (jp + jj + 1) * 128)
                        for half in range(2):
                            qs = slice(q0 + half * 512, q0 + (half + 1) * 512)
                            nc.tensor.matmul(
                                scp[jj][:, half * 512:(half + 1) * 512],
                                kt[h][:, js], qt[h][:, qs],
                                start=True, stop=False)
                    # the 4 leftover K=32 matmuls of the pair run on the 4
                    # distinct 32-row strips (4 distinct PSUM banks) -> all
                    # concurrent on the PE
                    for bi in range(4):
                        jj, half = bi // 2, bi % 2
                        s = 32 * bi
                        use2 = (bi + h) % 2 == 1
                        ktile = ((qkd2 if use2 else qkd) if bi < 2
                                 else (qkb2 if use2 else qkb))
                        qtile = ((qkb2 if use2 else qkb) if bi < 2
                                 else (qkd2 if use2 else qkd))
                        js = slice((jp + jj) * 128, (jp + jj + 1) * 128)
                        qs = slice(q0 + half * 512, q0 + (half + 1) * 512)
                        nc.tensor.matmul(
                            scp[jj][:, half * 512:(half + 1) * 512],
                            ktile[s:s + 32, js], qtile[s:s + 32, qs],
                            start=False, stop=True, tile_position=(s, 0))
                    for jj in range(2):
                        ex = exp_.tile([128, QC], BF16, tag="ex")
                        nc.scalar.activation(ex[:], scp[jj][:], EXP)
                        exs[(h, jp + jj)] = ex

                def emit_pv(h, jp):
                    for jj in range(2):
                        j = jp + jj
                        ex = exs.pop((h, j))
                        for ql in range(8):
                            # start=True clears has_written BANK-wide: only
                            # the first matmul per bank may use it.
                            st = (j == 0) and ql in (0, 3, 6)
                            nc.tensor.matmul(
                                pv_slot(pvt, ql),
                                ex[:, ql * 128:(ql + 1) * 128],
                                V[h][:, j, :], start=st, stop=(j == NJ - 1),
                                skip_group_check=True)

                natA = {}
                natN = [nnp.tile([128, 64], F32, tag=f"nn{ql}",
                                 name=f"nn{ql}")
                        for ql in range(8)]

                def emit_evac(h):
                    for ql in range(8):
                        pv = pv_slot(pvt, ql)
                        rec = rcp.tile([128, 1], F32, tag=f"rc{h}_{ql}",
                                       name="rec")
                        nc.vector.reciprocal(rec[:], pv[:, 160:161])
                        na = nap.tile([128, 128], F32, tag=f"na{h}_{ql}",
                                      name="na")
                        nc.vector.tensor_scalar_mul(na[:], pv[:, 0:128],
                                                    rec[:])
                        nc.vector.tensor_scalar_mul(
                            natN[ql][:, 32 * h:32 * h + 32],
                            pv[:, 128:160], rec[:])
                        natA[(h, ql)] = na

                emit_scores(*pairs[0])
                if pending is not None:
                    emit_tail(*pending, range(0, 4))
                for i, (h, jp) in enumerate(pairs):
                    if i + 1 < len(pairs):
                        emit_scores(*pairs[i + 1])
                    emit_pv(h, jp)
                    if jp == NJ - 2:
                        emit_evac(h)
                        if h == 0 and pending is not None:
                            emit_tail(*pending, range(4, 8))
                pending = (natA, natN, q0)
            emit_tail(*pending, range(0, 8))

    nc.compile()
    return nc


def kernel(hidden_states, w_q, w_k, w_v, lora_k_a, lora_k_b,
           lora_v_a, lora_v_b, w_out, b_out):
    f64 = np.float64
    bf16 = ml_dtypes.bfloat16
    wk_eff = (w_k.astype(f64)
              + w_k.astype(f64) @ lora_k_a.astype(f64) @ lora_k_b.astype(f64)
              ).astype(np.float32)
    wv_eff = (w_v.astype(f64)
              + w_v.astype(f64) @ lora_v_a.astype(f64) @ lora_v_b.astype(f64)
              ).astype(np.float32)
    wq_s = (w_q.astype(f64) / np.sqrt(DH)).astype(np.float32)

    ident = np.eye(128, dtype=np.float32)
    xT = [np.ascontiguousarray(np.asarray(hidden_states)[b].T).astype(bf16)
          for b in range(B)]

    in_maps = []
    for c in range(N_CORES):
        b, p = c // 4, c % 4
        ha, hb = 2 * p, 2 * p + 1
        mainq = np.concatenate([np.arange(ha * DH, ha * DH + 128),
                                np.arange(hb * DH, hb * DH + 128)])
        left = np.concatenate([np.arange(ha * DH + 128, (ha + 1) * DH),
                               np.arange(hb * DH + 128, (hb + 1) * DH)])
        # wo rows follow the oT layout: [h0 d0:128 | h1 d0:128 | leftovers]
        perm = np.concatenate([mainq, left])
        cols = slice(p * HP, (p + 1) * HP)
        in_maps.append({
            "xT": xT[b],
            "wq": np.ascontiguousarray(wq_s[:, mainq]).astype(bf16),
            "wk": np.ascontiguousarray(wk_eff[:, mainq]).astype(bf16),
            "wqk": np.ascontiguousarray(
                np.concatenate([wq_s[:, left], wk_eff[:, left]],
                               axis=1)).astype(bf16),
            "wv": np.ascontiguousarray(wv_eff[:, cols]).astype(bf16),
            "wo": np.ascontiguousarray(w_out[perm, :]).astype(bf16),
            "ident": ident,
        })

    global _last_in_maps
    _last_in_maps = in_maps
    if "nc" not in _CACHE:
        _CACHE["nc"] = build()
    res = run_bass_kernel_spmd(_CACHE["nc"], in_maps, list(range(N_CORES)))

    out = np.zeros((B, S, D), np.float32)
    for c in range(N_CORES):
        out[c // 4] += res.results[c]["out"]
    out += np.asarray(b_out, np.float32)
    return out


# revision 28
# speedup vs baseline: 1.0143x; 1.0046x over previous
"""LoRA attention processor on 8 NeuronCores (Trainium2, Bass/Tile), bf16.

Reference computation (B=2, S=4096, D=1280, H=8 heads, dh=160, rank-4 LoRA
on K/V):
    q = x @ Wq; k = x @ Wk; v = x @ Wv
    k += (k @ Ak) @ Bk; v += (v @ Av) @ Bv        (LoRA, rank 4)
    attn = softmax(q k^T / sqrt(dh)) v   per head
    out = attn @ Wout + b_out

Sharding: core c handles batch b = c//4 and head pair p = c%4 (columns
320p:320p+320 of the QKV projections, rows of Wout). LoRA is folded into
the weights on the host. Each core returns a partial output (its heads'
contribution to attn@Wout); the host sums the 4 partials per batch and
adds the bias.

Design notes (all empirically driven; the PE gets power-clamped to
1.2 GHz after ~220us of sustained 8-core matmul activity, so the kernel
is column-count-bound):
- All matmuls in bf16 (fp32r keeps the clamp at ~50% util for the whole
  run); everything SBUF-resident.
- Projections: per-head d0:128 chunks of Q and K are produced as [128,S]
  tiles; the two heads' leftover d128:160 dims of BOTH q and k are fused
  into one 128-wide M-chunk (host column permutation), then a SBUF->SBUF
  DMA builds a half-swapped duplicate so that k3/q3 coexist in strips
  {32h} and {64+32h} - the two K=32 leftover score matmuls of a q-chunk
  then run CONCURRENTLY on disjoint 32-row PE tiles (different PSUM
  banks; same-bank concurrency is a hardware fault).
- Scores are computed transposed ([k,q]) in 1024-wide q chunks, j-pairs
  batched so 128-row-mode and 32-row-mode matmuls alternate once per j
  instead of twice (mode switches drain the PE). exp runs on ACT over
  [128,1024] (2 PSUM banks per instruction, halving ACT overhead).
- PV runs in natural layout out[q,d]: lhsT = exp-tile slices, rhs =
  V[k-block] with a ones-column appended (denominator rides along as
  column 160). PSUM start=True clears has_written bits BANK-wide, so only
  the first matmul of each bank uses start=True; the other interleaved
  accumulation groups in that bank begin with start=False (cleared bits
  -> overwrite) - verified on hardware.
- Normalization is folded into PSUM evacuation (ACT copy with
  per-partition 1/denom scale), the normalized tiles are transposed on
  the PE, and the output projection contracts d in (128,128,64) chunks.
"""

import numpy as np
import ml_dtypes
from contextlib import ExitStack

import concourse.bass as bass
import concourse.tile as tile
from concourse import bacc, mybir
from concourse.bass_utils import run_bass_kernel_spmd

B, S, D = 2, 4096, 1280
H, DH = 8, 160
HP = 320           # head-pair columns per core (2 heads)
N_CORES = 8
SC = 512           # phase-1 free-dim chunk
NSC = S // SC      # 8
QC = 1024          # phase-2 q chunk (2 PSUM banks wide)
NQC = S // QC      # 4
CK = 128           # contraction chunk
NCK = D // CK      # 10
NJ = S // 128      # 32 k-blocks
F32 = mybir.dt.float32
BF16 = mybir.dt.bfloat16
EXP = mybir.ActivationFunctionType.Exp

CHUNKS = [(0, 128), (128, 128), (256, 64)]   # oT / wo row chunks

_CACHE = {}


def build():
    nc = bacc.Bacc("TRN2", target_bir_lowering=False, debug=False,
                   num_devices=N_CORES)
    xT = nc.dram_tensor("xT", [D, S], BF16, kind="ExternalInput").ap()
    wq = nc.dram_tensor("wq", [D, 256], BF16, kind="ExternalInput").ap()
    wk = nc.dram_tensor("wk", [D, 256], BF16, kind="ExternalInput").ap()
    wqk = nc.dram_tensor("wqk", [D, 128], BF16, kind="ExternalInput").ap()
    wv = nc.dram_tensor("wv", [D, HP], BF16, kind="ExternalInput").ap()
    wo = nc.dram_tensor("wo", [HP, D], BF16, kind="ExternalInput").ap()
    ident = nc.dram_tensor("ident", [128, 128], F32, kind="ExternalInput").ap()
    out = nc.dram_tensor("out", [S, D], F32, kind="ExternalOutput").ap()

    with tile.TileContext(nc) as tc, ExitStack() as top:
        # persistent SBUF tensors
        per = top.enter_context(tc.tile_pool(name="per", bufs=1))
        qt = [per.tile([128, S], BF16, name=f"qt{i}", tag=f"qt{i}")
              for i in range(2)]
        kt = [per.tile([128, S], BF16, name=f"kt{i}", tag=f"kt{i}")
              for i in range(2)]
        # leftover-dim tiles; rows hold [q3 h0|q3 h1|k3 h0|k3 h1] and the
        # swapped/head-swapped variants so each head's (k3, q3) pair exists
        # in all four 32-partition strips (4-way concurrent K=32 matmuls)
        qkb = per.tile([128, S], BF16, name="qkb", tag="qkb")
        qkd = per.tile([128, S], BF16, name="qkd", tag="qkd")
        qkb2 = per.tile([128, S], BF16, name="qkb2", tag="qkb2")
        qkd2 = per.tile([128, S], BF16, name="qkd2", tag="qkd2")
        V = [per.tile([128, NJ, 162], BF16, name=f"V{h}", tag=f"V{h}")
             for h in range(2)]
        oT = [per.tile([sz, S], BF16, name=f"oT{i}", tag=f"oT{i}")
              for i, (_, sz) in enumerate(CHUNKS)]
        id_t = per.tile([128, 128], F32, name="id_t", tag="id_t")

        # ---- phase 1: projections Q/K (transposed chunks) + V (natural) ----
        with ExitStack() as ph1:
            xp = ph1.enter_context(tc.tile_pool(name="xp", bufs=2))
            wp = ph1.enter_context(tc.tile_pool(name="wp", bufs=1))
            pp = ph1.enter_context(tc.tile_pool(name="pp", bufs=4, space="PSUM"))
            sp = ph1.enter_context(tc.tile_pool(name="sp", bufs=2))

            warm = sp.tile([1, 2], F32, tag="warm")
            nc.vector.memset(warm[:], 0.0)
            warm2 = sp.tile([1, 2], F32, tag="warm2")
            nc.scalar.activation(warm2[:], warm[:], EXP)

            # one strided DMA per weight tensor: dram rows c*128+p ->
            # partition p, chunk c of a [128, NCK, w] tile. wq and the first
            # x chunk are issued first so the very first matmul can start.
            wts = {}
            xt0 = None
            for nm, srcw, w_ in (("wq", wq, 256), ("wk", wk, 256),
                                 ("wqk", wqk, 128), ("wv", wv, HP)):
                t = wp.tile([128, NCK, w_], BF16, name=f"w_{nm}",
                            tag=f"w_{nm}")
                nc.sync.dma_start(
                    t[:], srcw.rearrange("(c p) w -> p c w", p=CK))
                for c in range(NCK):
                    wts[(nm, c)] = t[:, c]
                if nm == "wq":
                    xt0 = xp.tile([CK, NCK, SC], BF16, tag="xt", name="xt")
                    nc.sync.dma_start(
                        xt0[:], xT.rearrange("(c p) s -> p c s",
                                             p=CK)[:, :, 0:SC])
            nc.sync.dma_start(id_t[:], ident[:])
            for h in range(2):
                nc.vector.memset(V[h][:, :, 160:162], 0.0)
                nc.vector.memset(V[h][:, :, 160:161], 1.0)

            for sc in range(NSC):
                ss = slice(sc * SC, (sc + 1) * SC)
                if sc == 0:
                    xt = xt0
                else:
                    xt = xp.tile([CK, NCK, SC], BF16, tag="xt", name="xt")
                    nc.sync.dma_start(
                        xt[:], xT.rearrange("(c p) s -> p c s",
                                            p=CK)[:, :, ss])
                xts = [xt[:, c] for c in range(NCK)]
                # transposed projections: psum[m, s] = w[c, m].T @ xT[c, s]
                groups = [("wq", 0, qt[0]), ("wq", 128, qt[1]),
                          ("wk", 0, kt[0]), ("wk", 128, kt[1]),
                          ("wqk", 0, qkb)]
                for nm, off, dst in groups:
                    ps = pp.tile([128, SC], F32, tag="ps")
                    for c in range(NCK):
                        nc.tensor.matmul(
                            ps[:], wts[(nm, c)][:, off:off + 128], xts[c][:],
                            start=(c == 0), stop=(c == NCK - 1))
                    nc.vector.tensor_copy(dst[:, ss], ps[:])
                    if dst is qkb:
                        # duplicates via SBUF->SBUF DMA (shifts partitions),
                        # chunk-wise so they overlap phase-1 compute:
                        # qkb  = [q3h0|q3h1|k3h0|k3h1]   (produced)
                        # qkd  = [k3h0|k3h1|q3h0|q3h1]   (half swap)
                        # qkb2 = [q3h1|q3h0|k3h1|k3h0]   (head swap)
                        # qkd2 = [k3h1|k3h0|q3h1|q3h0]   (both)
                        nc.sync.dma_start(qkd[0:64, ss], qkb[64:128, ss])
                        nc.sync.dma_start(qkd[64:128, ss], qkb[0:64, ss])
                        for dst2, srcs in ((qkb2, (32, 0, 96, 64)),
                                           (qkd2, (96, 64, 32, 0))):
                            for r, sr in enumerate(srcs):
                                nc.sync.dma_start(
                                    dst2[32 * r:32 * r + 32, ss],
                                    qkb[sr:sr + 32, ss])
                # V natural: psum[s, dv] = xT[c, s].T @ wv[c, :]
                for st4 in range(4):
                    s0 = sc * 4 + st4
                    ps = pp.tile([128, HP], F32, tag="psv")
                    for c in range(NCK):
                        nc.tensor.matmul(
                            ps[:], xts[c][:, st4 * 128:(st4 + 1) * 128],
                            wts[("wv", c)][:], start=(c == 0),
                            stop=(c == NCK - 1))
                    for h in range(2):
                        nc.vector.tensor_copy(V[h][:, s0, 0:160],
                                              ps[:, h * 160:(h + 1) * 160])



        # ---- phase 2+3: attention + output projection, per 1024-q chunk ----
        # PSUM budget (8 banks): "sc"-tagged [128,1024]f32 tiles (2 banks x
        # 2 bufs = 4) host the score matmuls AND (via disjoint slices) the
        # transpose outputs and phase-3 accumulators; pv tiles take 3 banks.
        with ExitStack() as ph2:
            big = ph2.enter_context(tc.tile_pool(name="big", bufs=2,
                                                 space="PSUM"))
            pvp = ph2.enter_context(tc.tile_pool(name="pvp", bufs=1,
                                                 space="PSUM"))
            ptp = ph2.enter_context(tc.tile_pool(name="ptp", bufs=1,
                                                 space="PSUM"))
            exp_ = ph2.enter_context(tc.tile_pool(name="exq", bufs=6))
            nap = ph2.enter_context(tc.tile_pool(name="nap", bufs=2))
            nnp = ph2.enter_context(tc.tile_pool(name="nnp", bufs=2))
            rcp = ph2.enter_context(tc.tile_pool(name="rcp", bufs=2))
            wop = ph2.enter_context(tc.tile_pool(name="wop", bufs=1))
            fsp = ph2.enter_context(tc.tile_pool(name="fsp", bufs=2))

            woc = []
            for i, (off, msz) in enumerate(CHUNKS):
                w = wop.tile([msz, D], BF16, name=f"wo{i}", tag=f"wo{i}")
                nc.sync.dma_start(w[:], wo[off:off + msz, :])
                woc.append(w)

            # pv psum slot for a q-subtile (0..7): 3+3+2 per bank
            def pv_slot(pvt, ql):
                if ql < 3:
                    return pvt[0][:, ql]
                if ql < 6:
                    return pvt[1][:, ql - 3]
                return pvt[2][:, ql - 6]

            def emit_tail(natA, natN, q0, qls):
                # per q-subtile: transpose natural [q, d] -> oT chunks [d, q]
                # (f32 PE transpose mode, own 1-bank pool), then immediately
                # the output projection for that 128-row block. Emitted in
                # two halves at the points where the PE would otherwise stall
                # on the ACT evacuation chain (next chunk's start and its
                # h0->h1 transition).
                for ql in qls:
                    qg = slice(q0 + ql * 128, q0 + (ql + 1) * 128)
                    pt = ptp.tile([128, 3, 128], F32, tag="pt", name="pt")
                    nc.tensor.transpose(pt[:, 0], natA[(0, ql)][:], id_t[:])
                    nc.tensor.transpose(pt[:, 1], natA[(1, ql)][:], id_t[:])
                    nc.tensor.transpose(pt[0:64, 2], natN[ql][:], id_t[:])
                    nc.vector.tensor_copy(oT[0][:, qg], pt[:, 0])
                    nc.vector.tensor_copy(oT[1][:, qg], pt[:, 1])
                    nc.vector.tensor_copy(oT[2][:, qg], pt[0:64, 2])
                    row = q0 + ql * 128
                    sbs = slice(row, row + 128)
                    fs = fsp.tile([128, D], F32, tag="fs", name="fs")
                    for oi, (oc, osz) in enumerate(((0, 512), (512, 512),
                                                    (1024, 256))):
                        ps = big.tile([128, QC], F32, tag="sc", name="fo")
                        for i in range(3):
                            nc.tensor.matmul(ps[:, 0:osz], oT[i][:, sbs],
                                             woc[i][:, oc:oc + osz],
                                             start=(i == 0), stop=(i == 2))
                        if oi % 2 == 0:
                            nc.vector.tensor_copy(fs[:, oc:oc + osz],
                                                  ps[:, 0:osz])
                        else:
                            nc.scalar.copy(fs[:, oc:oc + osz], ps[:, 0:osz])
                        nc.sync.dma_start(out[sbs, oc:oc + osz],
                                          fs[:, oc:oc + osz])

            pending = None

            for qc in range(NQC):
                q0 = qc * QC
                pairs = [(h, jp) for h in range(2) for jp in range(0, NJ, 2)]
                pvt = [pvp.tile([128, 3, 162], F32, tag="pv0", name="pv0"),
                       pvp.tile([128, 3, 162], F32, tag="pv1", name="pv1"),
                       pvp.tile([128, 2, 162], F32, tag="pv2", name="pv2")]
                exs = {}

                def emit_scores(h, jp):
                    """Scores for j-pair (jp, jp+1): 128-mode matmuls batched
                    before 32-mode ones (1 mode switch per j instead of 2);
                    the two K=32 leftovers of each j run concurrently on
                    disjoint 32-row tiles / different PSUM banks."""
                    scp = [big.tile([128, QC], F32, tag="sc", name="sca"),
                           big.tile([128, QC], F32, tag="sc", name="scb")]
                    for jj in range(2):
                        js = slice((jp + jj) * 128, (jp + jj + 1) * 128)
                        for half in range(2):
                            qs = slice(q0 + half * 512, q0 + (half + 1) * 512)
                            nc.tensor.matmul(
                                scp[jj][:, half * 512:(half + 1) * 512],
                                kt[h][:, js], qt[h][:, qs],
                                start=True, stop=False)
                    # the 4 leftover K=32 matmuls of the pair run on the 4
                    # distinct 32-row strips (4 distinct PSUM banks) -> all
                    # concurrent on the PE
                    for bi in range(4):
                        jj, half = bi // 2, bi % 2
                        s = 32 * bi
                        use2 = (bi + h) % 2 == 1
                        ktile = ((qkd2 if use2 else qkd) if bi < 2
                                 else (qkb2 if use2 else qkb))
                        qtile = ((qkb2 if use2 else qkb) if bi < 2
                                 else (qkd2 if use2 else qkd))
                        js = slice((jp + jj) * 128, (jp + jj + 1) * 128)
                        qs = slice(q0 + half * 512, q0 + (half + 1) * 512)
                        nc.tensor.matmul(
                            scp[jj][:, half * 512:(half + 1) * 512],
                            ktile[s:s + 32, js], qtile[s:s + 32, qs],
                            start=False, stop=True, tile_position=(s, 0))
                    for jj in range(2):
                        ex = exp_.tile([128, QC], BF16, tag="ex")
                        nc.scalar.activation(ex[:], scp[jj][:], EXP)
                        exs[(h, jp + jj)] = ex

                def emit_pv(h, jp):
                    for jj in range(2):
                        j = jp + jj
                        ex = exs.pop((h, j))
                        for ql in range(8):
                            # start=True clears has_written BANK-wide: only
                            # the first matmul per bank may use it.
                            st = (j == 0) and ql in (0, 3, 6)
                            nc.tensor.matmul(
                                pv_slot(pvt, ql),
                                ex[:, ql * 128:(ql + 1) * 128],
                                V[h][:, j, :], start=st, stop=(j == NJ - 1),
                                skip_group_check=True)

                natA = {}
                natN = [nnp.tile([128, 64], F32, tag=f"nn{ql}",
                                 name=f"nn{ql}")
                        for ql in range(8)]

                def emit_evac(h):
                    for ql in range(8):
                        pv = pv_slot(pvt, ql)
                        rec = rcp.tile([128, 1], F32, tag=f"rc{h}_{ql}",
                                       name="rec")
                        nc.vector.reciprocal(rec[:], pv[:, 160:161])
                        na = nap.tile([128, 128], F32, tag=f"na{h}_{ql}",
                                      name="na")
                        nc.vector.tensor_scalar_mul(na[:], pv[:, 0:128],
                                                    rec[:])
                        nc.vector.tensor_scalar_mul(
                            natN[ql][:, 32 * h:32 * h + 32],
                            pv[:, 128:160], rec[:])
                        natA[(h, ql)] = na

                emit_scores(*pairs[0])
                emit_scores(*pairs[1])
                if pending is not None:
                    emit_tail(*pending, range(0, 4))
                for i, (h, jp) in enumerate(pairs):
                    if i + 2 < len(pairs):
                        emit_scores(*pairs[i + 2])
                    emit_pv(h, jp)
                    if jp == NJ - 2:
                        emit_evac(h)
                        if h == 0 and pending is not None:
                            emit_tail(*pending, range(4, 8))
                pending = (natA, natN, q0)
            emit_tail(*pending, range(0, 8))

    nc.compile()
    return nc


def kernel(hidden_states, w_q, w_k, w_v, lora_k_a, lora_k_b,
           lora_v_a, lora_v_b, w_out, b_out):
    f64 = np.float64
    bf16 = ml_dtypes.bfloat16
    wk_eff = (w_k.astype(f64)
              + w_k.astype(f64) @ lora_k_a.astype(f64) @ lora_k_b.astype(f64)
              ).astype(np.float32)
    wv_eff = (w_v.astype(f64)
              + w_v.astype(f64) @ lora_v_a.astype(f64) @ lora_v_b.astype(f64)
              ).astype(np.float32)
    wq_s = (w_q.astype(f64) / np.sqrt(DH)).astype(np.float32)

    ident = np.eye(128, dtype=np.float32)
    xT = [np.ascontiguousarray(np.asarray(hidden_states)[b].T).astype(bf16)
          for b in range(B)]

    in_maps = []
    for c in range(N_CORES):
        b, p = c // 4, c % 4
        ha, hb = 2 * p, 2 * p + 1
        mainq = np.concatenate([np.arange(ha * DH, ha * DH + 128),
                                np.arange(hb * DH, hb * DH + 128)])
        left = np.concatenate([np.arange(ha * DH + 128, (ha + 1) * DH),
                               np.arange(hb * DH + 128, (hb + 1) * DH)])
        # wo rows follow the oT layout: [h0 d0:128 | h1 d0:128 | leftovers]
        perm = np.concatenate([mainq, left])
        cols = slice(p * HP, (p + 1) * HP)
        in_maps.append({
            "xT": xT[b],
            "wq": np.ascontiguousarray(wq_s[:, mainq]).astype(bf16),
            "wk": np.ascontiguousarray(wk_eff[:, mainq]).astype(bf16),
            "wqk": np.ascontiguousarray(
                np.concatenate([wq_s[:, left], wk_eff[:, left]],
                               axis=1)).astype(bf16),
            "wv": np.ascontiguousarray(wv_eff[:, cols]).astype(bf16),
            "wo": np.ascontiguousarray(w_out[perm, :]).astype(bf16),
            "ident": ident,
        })

    global _last_in_maps
    _last_in_maps = in_maps
    if "nc" not in _CACHE:
        _CACHE["nc"] = build()
    res = run_bass_kernel_spmd(_CACHE["nc"], in_maps, list(range(N_CORES)))

    out = np.zeros((B, S, D), np.float32)
    for c in range(N_CORES):
        out[c // 4] += res.results[c]["out"]
    out += np.asarray(b_out, np.float32)
    return out
